# revision 1
# baseline (speedup 1.0000x reference)
"""Trainium2 Bass kernel for nn_ProtoCycleModel (retrieval_knn).

Problem: P=65536 prototypes, C=64 classes, D=256.
Per class c (rows c::64 of each table, n=1024):
    p2_inv = (p2_c - b) @ inv(W.T)          # y-side of direction "source"
    p1_fwd = p1_c @ W.T + b                 # y-side of direction "target"
    loss_src[c] = mean_i min_j ||p1_c[i] - p2_inv[j]||^2
    loss_tgt[c] = mean_i min_j ||p2_c[i] - p1_fwd[j]||^2
Output: (2, 64) fp32.

Sharding: class axis across 8 cores (8 classes/core). Each core:
  - loads its (8*1024, 256) slices of both tables (row-major, contiguous)
  - PE-transposes them to d-major (fp32 exact)
  - computes transformed tables directly in transposed space:
        yT = Mat @ xT + bias   (Mat = -2*inv(W.T)-style, folded scale -2)
    so the pairwise matmul G = xT.T @ yT gives -2 * x.y' directly.
  - |y'|^2 row: ones-matmul over Square(transform psum + bias) (scale 1/4
    baked into a 0.25-constant stationary matrix), broadcast to all 128
    partitions for free via M=128 stationary ones.
  - per i-tile: fused DVE tensor_tensor_reduce: min_j (G + |y'|^2) -> [128,1]
  - per-class scalars via ones-matmul cross-partition sum; host gathers.

All matmuls run in float32r (TF32-like, full PE rate at N>=512, ~16x more
accurate than bf16). Everything else fp32.
"""

import numpy as np

P, C, D = 65536, 64, 256
N_CORES = 8
CPC = C // N_CORES          # classes per core = 8
NPC = P // C                # prototypes per class = 1024
IT = NPC // 128             # i-tiles per class = 8

# ys application mode: "ttr" = fused DVE tensor_tensor_reduce;
# "fold" = K=1 matmul folds ys row into PSUM, then plain tensor_reduce.
YS_MODE = "fold"
import os as _os
PSG_WIDE = _os.environ.get("K_PSG_WIDE", "0") == "1"   # [128,1024] G tiles
PSG_BUFS = int(_os.environ.get("K_PSG_BUFS", "4"))
PSM_BUFS = int(_os.environ.get("K_PSM_BUFS", "2"))

_CACHE = {}


def _build_bass():
    import concourse.bass as bass
    from concourse import bacc
    import concourse.tile as tile
    from concourse import mybir
    from concourse.masks import make_identity

    FP32 = mybir.dt.float32
    FP32R = mybir.dt.float32r
    BF16 = mybir.dt.bfloat16
    AF = mybir.ActivationFunctionType
    ALU = mybir.AluOpType
    AX = mybir.AxisListType

    nc = bacc.Bacc(None, target_bir_lowering=False)

    p1_d = nc.dram_tensor("p1", [CPC * NPC, D], FP32, kind="ExternalInput")
    p2_d = nc.dram_tensor("p2", [CPC * NPC, D], FP32, kind="ExternalInput")
    # mats[dir][kchunk] : [128, 256] fp32, lhsT layout [d, d'] with the -2
    # scale folded in.  dir 0 = source (V2 = -2*inv(W.T)), dir 1 = target
    # (Wt2 = -2*W.T).
    mats_d = nc.dram_tensor("mats", [2, 2, 128, D], FP32, kind="ExternalInput")
    consts_d = nc.dram_tensor("consts", [128, 385], FP32, kind="ExternalInput")
    # biases[dir] : [128, 2] fp32 (column = d' chunk);  dir0 = +2*(b@V),
    # dir1 = -2*b.
    bias_d = nc.dram_tensor("biases", [2, 128, 2], FP32, kind="ExternalInput")
    out_d = nc.dram_tensor("out", [1, 2 * CPC], FP32, kind="ExternalOutput")

    with tile.TileContext(nc) as tc:
        with (
            tc.tile_pool(name="const", bufs=1) as const,
            tc.tile_pool(name="xrow", bufs=6) as xrow_p,
            tc.tile_pool(name="xt", bufs=10) as xt_p,
            tc.tile_pool(name="yt", bufs=8) as yt_p,
            tc.tile_pool(name="sq", bufs=4) as sq_p,
            tc.tile_pool(name="ysb", bufs=4) as ysb_p,
            tc.tile_pool(name="scr", bufs=3) as scr_p,
            tc.tile_pool(name="psg", bufs=PSG_BUFS, space="PSUM") as psg_p,
            tc.tile_pool(name="psm", bufs=PSM_BUFS, space="PSUM") as psm_p,
        ):
            # ---- constants ----
            cblk_raw = const.tile([128, 385], FP32)
            nc.scalar.dma_start(cblk_raw[:], consts_d[:])
            cblk = const.tile([128, 385], FP32R)
            nc.vector.tensor_copy(cblk[:], cblk_raw[:])

            mats_raw = const.tile([128, 2, 2, D], FP32)
            nc.scalar.dma_start(mats_raw[:], mats_d[:].rearrange("a b p d -> p a b d"))
            mats = const.tile([128, 2, 2, D], FP32R)
            nc.vector.tensor_copy(mats[:], mats_raw[:])

            biases = const.tile([128, 2, 2], FP32)  # [p, dir, dchunk]
            nc.scalar.dma_start(biases[:], bias_d[:].rearrange("a p c -> p a c"))
            identr = cblk[:, 0:128]
            identf = cblk_raw[:, 0:128]
            ones1r = cblk[:, 128:129]
            ones_q = cblk[:, 257:385]

            pmin = const.tile([128, 2 * CPC * IT], FP32)   # col = dir*64+c*8+it
            pmin2 = (const.tile([128, 2 * CPC * IT], FP32, name="pmin2")
                     if not PSG_WIDE else pmin)
            pxs = const.tile([128, 2 * CPC * 2], FP32)     # col = dir*16+c*2+dc

            onesrow = cblk[0:1, 128:256]

            # ---- main loop: software-pipelined (prep one class ahead) ----
            state = {}

            def prep(c):
                xts = [[None, None], [None, None]]  # [table][dchunk]
                for t in range(2):
                    src_d = p1_d if t == 0 else p2_d
                    xr = xrow_p.tile([128, IT, D], FP32, tag="xrow", bufs=3)
                    xrr = xrow_p.tile([128, IT, D], FP32R, tag="xrowr", bufs=4)
                    half = NPC // 2
                    for hh in range(2):
                        nc.sync.dma_start(
                            xr[:, hh * (IT // 2):(hh + 1) * (IT // 2), :],
                            src_d[c * NPC + hh * half:
                                  c * NPC + (hh + 1) * half, :].rearrange(
                                "(k p) d -> p k d", p=128),
                        )
                        nc.vector.tensor_copy(
                            xrr[:, hh * (IT // 2):(hh + 1) * (IT // 2), :],
                            xr[:, hh * (IT // 2):(hh + 1) * (IT // 2), :])
                    for dc in range(2):
                        pst = psm_p.tile([128, 1024], FP32R, tag="misc")
                        for k in range(IT):
                            nc.tensor.transpose(
                                pst[:, k * 128:(k + 1) * 128],
                                xrr[:, k, dc * 128:(dc + 1) * 128],
                                identr,
                            )
                        xt_t = xt_p.tile([128, NPC], FP32R, tag="xt")
                        nc.scalar.copy(xt_t[:], pst[:])
                        xts[t][dc] = xt_t
                        # xs partials: sum_i x^2 per d-partition
                        trash = scr_p.tile([128, NPC], BF16, tag="scr")
                        nc.scalar.activation(
                            trash[:], xt_t[:], AF.Square,
                            accum_out=pxs[:, t * 16 + c * 2 + dc:
                                          t * 16 + c * 2 + dc + 1],
                        )

                yts_all = [[], []]
                ysrow_all = [None, None]
                for dr in range(2):
                    ysrc = xts[1 - dr]    # dir0: y from p2; dir1: y from p1
                    sqs = []
                    for dcp in range(2):   # output d' chunk
                        pstf = psm_p.tile([128, 1024], FP32, tag="misc")
                        for dc in range(2):
                            for ih in range(2):
                                nc.tensor.matmul(
                                    pstf[:, ih * 512:(ih + 1) * 512],
                                    mats[:, dr, dc, dcp * 128:(dcp + 1) * 128],
                                    ysrc[dc][:, ih * 512:(ih + 1) * 512],
                                    start=(dc == 0), stop=(dc == 1),
                                )
                        bias_ap = biases[:, dr, dcp:dcp + 1]
                        yt_t = yt_p.tile([128, NPC], FP32R, tag="yt")
                        nc.scalar.activation(
                            yt_t[:], pstf[:], AF.Identity, bias=bias_ap, scale=1.0)
                        sq_t = sq_p.tile([128, NPC], FP32R, tag="sq")
                        nc.scalar.activation(
                            sq_t[:], pstf[:], AF.Square, bias=bias_ap, scale=1.0)
                        yts_all[dr].append(yt_t)
                        sqs.append(sq_t)

                    psy = psm_p.tile([128, 1024], FP32, tag="misc")
                    for jh in range(2):
                        for dcp in range(2):
                            nc.tensor.matmul(
                                psy[0:1, jh * 512:(jh + 1) * 512],
                                ones_q[:, 0:1],
                                sqs[dcp][:, jh * 512:(jh + 1) * 512],
                                start=(dcp == 0), stop=(dcp == 1),
                            )
                    ysrow = ysb_p.tile([1, NPC], FP32R, tag="ysrow")
                    nc.scalar.copy(ysrow[:], psy[0:1, :])
                    ysrow_all[dr] = ysrow
                state[c] = (xts, yts_all, ysrow_all)

            def pairwise(c):
                xts, yts_all, ysrow_all = state.pop(c)
                for dr in range(2):
                    xside = xts[dr]       # dir0: x = p1; dir1: x = p2
                    yts = yts_all[dr]
                    ysrow = ysrow_all[dr]
                    for it in range(IT):
                        col = dr * 64 + c * 8 + it
                        pgs = [psg_p.tile([128, 512], FP32, tag="g",
                                          name=f"pg{jh}")
                               for jh in range(2)]
                        for dc in range(2):          # stationary reused 2x
                            for jh in range(2):
                                nc.tensor.matmul(
                                    pgs[jh][:],
                                    xside[dc][:, it * 128:(it + 1) * 128],
                                    yts[dc][:, jh * 512:(jh + 1) * 512],
                                    start=(dc == 0), stop=False,
                                )
                        for jh in range(2):          # ys fold, ones stationary
                            nc.tensor.matmul(
                                pgs[jh][:],
                                onesrow,
                                ysrow[:, jh * 512:(jh + 1) * 512],
                                start=False, stop=True,
                            )
                        for jh in range(2):
                            dst = pmin if jh == 0 else pmin2
                            nc.vector.tensor_reduce(
                                out=dst[:, col:col + 1], in_=pgs[jh][:],
                                axis=AX.X, op=ALU.min,
                            )

            prep(0)
            for c in range(CPC):
                if c + 1 < CPC:
                    prep(c + 1)
                pairwise(c)

            # ---- finals ----
            if PSG_WIDE:
                pminc = pmin
            else:
                pminc = const.tile([128, 2 * CPC * IT], FP32, name="pminc")
                nc.vector.tensor_tensor(
                    out=pminc[:], in0=pmin[:], in1=pmin2[:], op=ALU.min)
            red_min = const.tile([128, 16], FP32)
            nc.vector.tensor_reduce(
                out=red_min[:], in_=pminc[:].rearrange("p (g k) -> p g k", k=IT),
                axis=AX.X, op=ALU.add)
            red_xs = const.tile([128, 16], FP32)
            nc.vector.tensor_reduce(
                out=red_xs[:], in_=pxs[:].rearrange("p (g k) -> p g k", k=2),
                axis=AX.X, op=ALU.add)
            red = const.tile([128, 16], FP32R)
            nc.vector.tensor_tensor(
                out=red[:], in0=red_min[:], in1=red_xs[:], op=ALU.add)
            psf = psm_p.tile([1, 16], FP32, tag="misc")
            nc.tensor.matmul(psf[:], ones1r, red[:], start=True, stop=True)
            outrow = const.tile([1, 16], FP32)
            nc.scalar.mul(outrow[:], psf[:], 1.0 / NPC)
            nc.sync.dma_start(out_d[:], outrow[:])

    nc.compile()
    return nc


def _get_nc():
    if "nc" not in _CACHE:
        _CACHE["nc"] = _build_bass()
    return _CACHE["nc"]


def kernel(protos1, protos2, W, b, num_classes):
    from concourse.bass_utils import run_bass_kernel_spmd

    nc_classes = int(num_classes)
    assert nc_classes == C and protos1.shape == (P, D)

    protos1 = np.ascontiguousarray(protos1, dtype=np.float32)
    protos2 = np.ascontiguousarray(protos2, dtype=np.float32)
    W = np.asarray(W, dtype=np.float32)
    b = np.asarray(b, dtype=np.float32)

    # host-side tiny prep: inverse + scaled transform matrices
    V = np.linalg.inv(W.T.astype(np.float64)).astype(np.float32)  # (p2-b)@V
    V2 = (-2.0 * V).astype(np.float32)                 # lhsT [d, d'] dir0
    Wt2 = (-2.0 * W.T).astype(np.float32)              # lhsT [d, d'] dir1
    bias0 = (2.0 * (b.astype(np.float64) @ V.astype(np.float64))).astype(np.float32)
    bias1 = (-2.0 * b).astype(np.float32)
    mats = np.stack([
        np.stack([V2[0:128, :], V2[128:256, :]]),
        np.stack([Wt2[0:128, :], Wt2[128:256, :]]),
    ]).astype(np.float32)                               # [2, 2, 128, 256]
    idb = np.eye(128, dtype=np.float32)
    consts = np.concatenate([
        idb,
        np.ones((128, 129), dtype=np.float32),
        np.full((128, 128), 0.25, dtype=np.float32),
    ], axis=1)
    biases = np.stack([
        bias0.reshape(2, 128).T,                        # [128, 2] cols = chunk
        bias1.reshape(2, 128).T,
    ]).astype(np.float32)                               # [2, 128, 2]

    # class-major reordering: (P, D) -> (C, NPC, D)
    p1c = np.ascontiguousarray(protos1.reshape(NPC, C, D).transpose(1, 0, 2))
    p2c = np.ascontiguousarray(protos2.reshape(NPC, C, D).transpose(1, 0, 2))

    in_maps = []
    for core in range(N_CORES):
        sl = slice(core * CPC, (core + 1) * CPC)
        in_maps.append({
            "p1": np.ascontiguousarray(p1c[sl].reshape(CPC * NPC, D)),
            "p2": np.ascontiguousarray(p2c[sl].reshape(CPC * NPC, D)),
            "mats": mats,
            "biases": biases,
            "consts": consts,
        })

    nc = _get_nc()
    res = run_bass_kernel_spmd(nc, in_maps, core_ids=list(range(N_CORES)))
    _CACHE["last_result"] = res

    out = np.zeros((2, C), dtype=np.float32)
    for core in range(N_CORES):
        row = res.results[core]["out"].reshape(2, CPC)
        out[:, core * CPC:(core + 1) * CPC] = row
    return out

